# revision 11
# baseline (speedup 1.0000x reference)
"""Dinov2 SDPA self-attention on one TRN2 chip (8 NeuronCores).

Problem: hidden_states [4, 1370, 1024], 16 heads x 64 dim, fp32.

Sharding (hybrid data/tensor parallel): core c handles batch b = c//2 and
head-group g = c%2 (8 heads = 512 hidden columns). Each core computes its
Q/K/V projections from its batch's activations and runs attention for its
8 heads; the host transposes/concatenates the per-core [512, 1370] ctx^T
outputs. No on-chip collectives needed.

Per-core algorithm (layouts transposed so softmax reductions become matmul
contractions; ALL outputs stay in transposed [dims, L] layout -- the final
[L, 512] transpose happens on the host during unshard, so no PE transposes):
  Xt = X^T in SBUF  [1024, 1370]
  Qt = Wq_g @ Xt + bq (per-partition bias)   [512, 1370]
  Kt = Wk_g @ Xt  (K bias is softmax-invariant -> dropped exactly)
  V  = X @ Wv_g^T + bv (natural layout; softmax weights sum to 1 so
       including bv here is exact)
  per head pair (hA, hB): ST = Kt_h^T-tiles @ Qt_h = scores^T [S, L]
       (contraction d=64, PE row groups 0-63/64-127)
  P^T = exp(ST/8) (ACT, fused 1/sqrt(d) scale; scores bounded ~|4|)
  "fat-ones" context: stationary for hA is [V_hA | 1*64], for hB is
       [1*64 | V_hB], so cAB[:,0] rows 0-63 = ctx^T_A, rows 64-127 =
       rowsums (64 replicated copies); cAB[:,1] mirrored. Normalization is
       then plain per-partition DVE work in the transposed layout:
       reciprocal of the sums half + tensor_mul into ostT. No transposes.

Schedule: input DMAs arrive per hidden-chunk and the Q/K/V projection
accumulators run k-major (8 concurrent PSUM banks) so the PE tracks DMA
arrival instead of serializing behind the full transfer. The attention
loop is chunk-outer with the next head-pair's Q/K projections emitted as
filler between score/ctx matmuls (keeps PE busy during exp waits), ctx
emission delayed 2 tiles behind exp so the single cAB accumulator bank
drains (DVE normalize) without stalling the next head pair, and ctx^T
output DMAs issued per (chunk, head-pair).

Matmul operands are bf16 (fp32 PSUM accumulation); normalize stays fp32.
Validated vs fp32 reference: ~3.5e-3 of absmax.
"""

import os

import numpy as np
import ml_dtypes

import concourse.mybir as mybir
import concourse.tile as tile
from concourse import bacc
from concourse import bass_utils

F32 = mybir.dt.float32
DT = mybir.dt.bfloat16
NPDT = ml_dtypes.bfloat16
AF = mybir.ActivationFunctionType

B = 4
L = 1370
HID = 1024
NH = 8            # heads per core
D = 64
QD = NH * D       # 512 projected dims per core
HP = NH // 2      # head pairs
KC = HID // 128   # contraction chunks for projections

L_CHUNKS = [(0, 512), (512, 512), (1024, 346)]
TILES = [(i * 128, min(128, L - i * 128)) for i in range((L + 127) // 128)]
NS = len(TILES)   # 11 (last tile 90)
CTX_DELAY = 3     # tiles of lag between exp and ctx emission
FILL_CAP = 17     # max proj-task steps pulled per attention window


def _body(nc, tc, xt_d, wq_d, wk_d, wv_d, bq_d, bv_d, out_d):
    with tc.tile_pool(name="persist", bufs=1) as pp:
        xt = pp.tile([128, KC, L], DT)
        wq = pp.tile([128, KC, QD], DT)
        wk = pp.tile([128, KC, QD], DT)
        wv = pp.tile([128, KC, QD], DT)
        qt = pp.tile([128, HP, L], DT)
        kt = pp.tile([128, HP, L], DT)
        vv = pp.tile([128, NS, HP, 2, 128], DT)  # fat stationary [V|1] / [1|V]
        ostT = pp.tile([128, HP, L], F32)        # ctx^T staging
        bqc = pp.tile([128, HP], F32)
        bvb = pp.tile([128, QD], F32)

        # ones halves of the fat stationaries (rowsum replication)
        nc.gpsimd.memset(vv[:, :, :, 0, D:], 1.0)
        nc.gpsimd.memset(vv[:, :, :, 1, 0:D], 1.0)

        # Input DMAs per hidden-chunk across the three DGE queues so the
        # k-major projection waves below track arrival.
        qs = [nc.sync, nc.scalar, nc.gpsimd]
        nc.gpsimd.dma_start(bqc[:, :], bq_d.rearrange("(h p) o -> p (h o)", p=128))
        nc.gpsimd.dma_start(bvb[:, :], bv_d[:, :])
        wqv = wq_d.rearrange("(k p) n -> p k n", p=128)
        wkv = wk_d.rearrange("(k p) n -> p k n", p=128)
        LH = 685
        for k in range(KC):
            r = slice(k * 128, (k + 1) * 128)
            qs[(5 * k) % 3].dma_start(xt[:, k, 0:LH], xt_d[r, 0:LH])
            qs[(5 * k + 1) % 3].dma_start(xt[:, k, LH:], xt_d[r, LH:])
            qs[(5 * k + 2) % 3].dma_start(wq[:, k, :], wqv[:, k, :])
            qs[(5 * k + 3) % 3].dma_start(wk[:, k, :], wkv[:, k, :])
            qs[(5 * k + 4) % 3].dma_start(wv[:, k, :], wv_d[r, :])

        def evict_v(vps, si, ss):
            for par in range(2):
                nc.vector.tensor_add(
                    vv[:ss, si, :, par:par + 1, par * D:par * D + D],
                    vps[:ss, :].rearrange("p (i two d) -> p i two d", two=2, d=D)[
                        :, :, par:par + 1, :],
                    bvb[:ss, :].rearrange("p (i two d) -> p i two d", two=2, d=D)[
                        :, :, par:par + 1, :],
                )

        def evict_q(qps, hp, l0, ln):
            nc.vector.tensor_scalar_add(qt[:, hp, l0:l0 + ln], qps[:, :ln],
                                        bqc[:, hp:hp + 1])

        def evict_k(kps, hp, l0, ln):
            nc.vector.tensor_copy(kt[:, hp, l0:l0 + ln], kps[:, :ln])

        # ---- phase A: k-major projection waves paced by the input DMAs ----
        with tc.tile_pool(name="proj", bufs=1, space="PSUM") as prp:
            # wave 1: V tiles 0-3 + Q/K hp0 chunks 0-1 (8 banks) advance
            # together chunk-by-chunk as xt/w chunks land.
            vps1 = [prp.tile([128, QD], F32, name="vps", tag=f"pa{t}")
                    for t in range(4)]
            qps1 = [prp.tile([128, 512], F32, name="qps", tag=f"pa{4 + c}")
                    for c in range(2)]
            kps1 = [prp.tile([128, 512], F32, name="kps", tag=f"pa{6 + c}")
                    for c in range(2)]
            for k in range(KC):
                st, sp = (k == 0), (k == KC - 1)
                for t in range(4):
                    s0, ss = TILES[t]
                    nc.tensor.matmul(vps1[t][:ss, :], xt[:, k, s0:s0 + ss],
                                     wv[:, k, :], start=st, stop=sp)
                for c in range(2):
                    l0, ln = L_CHUNKS[c]
                    nc.tensor.matmul(qps1[c][:, :ln], wq[:, k, 0:128],
                                     xt[:, k, l0:l0 + ln], start=st, stop=sp)
                for c in range(2):
                    l0, ln = L_CHUNKS[c]
                    nc.tensor.matmul(kps1[c][:, :ln], wk[:, k, 0:128],
                                     xt[:, k, l0:l0 + ln], start=st, stop=sp)
            for t in range(4):
                evict_v(vps1[t], t, TILES[t][1])
            for c in range(2):
                evict_q(qps1[c], 0, *L_CHUNKS[c])
                evict_k(kps1[c], 0, *L_CHUNKS[c])

            # wave 2: remaining V tiles + Q/K hp0 chunk 2 (xt fully resident)
            for t in range(4, NS):
                vps = prp.tile([128, QD], F32, name="vps", tag=f"pa{t - 4}")
                s0, ss = TILES[t]
                for k in range(KC):
                    nc.tensor.matmul(vps[:ss, :], xt[:, k, s0:s0 + ss],
                                     wv[:, k, :], start=(k == 0),
                                     stop=(k == KC - 1))
                evict_v(vps, t, ss)
            l0, ln = L_CHUNKS[2]
            qps = prp.tile([128, 512], F32, name="qps", tag="pa7")
            for k in range(KC):
                nc.tensor.matmul(qps[:, :ln], wq[:, k, 0:128],
                                 xt[:, k, l0:l0 + ln], start=(k == 0),
                                 stop=(k == KC - 1))
            evict_q(qps, 0, l0, ln)
            kps = prp.tile([128, 512], F32, name="kps", tag="pa0")
            for k in range(KC):
                nc.tensor.matmul(kps[:, :ln], wk[:, k, 0:128],
                                 xt[:, k, l0:l0 + ln], start=(k == 0),
                                 stop=(k == KC - 1))
            evict_k(kps, 0, l0, ln)

        # ---- phase B: attention with a deadline-ordered proj task queue ----
        # Projections for head pairs 1-3 run as PE filler between attention
        # matmuls, spread evenly across windows (~FILL_CAP steps each) so no
        # window leaves the PE starved while ACT streams exp. A chain's
        # results are emitted at least one full window before the window
        # whose scores read them (the deadline).
        with (
            tc.tile_pool(name="pqp", bufs=2, space="PSUM") as pqp,
            tc.tile_pool(name="sps", bufs=2, space="PSUM") as sps,
            tc.tile_pool(name="cps", bufs=1, space="PSUM") as cps,
            tc.tile_pool(name="wp", bufs=CTX_DELAY + 2) as wp,
            tc.tile_pool(name="wr", bufs=2) as wr,
        ):
            class Chain:
                def __init__(self, w, nhp, l0c, lnc, is_q):
                    self.w, self.nhp = w, nhp
                    self.l0c, self.lnc, self.is_q = l0c, lnc, is_q
                    self.ps = None

                def step(self, k):
                    if k == 0:
                        self.ps = pqp.tile([128, 512], F32, name="pqs",
                                           tag="pq")
                    nc.tensor.matmul(self.ps[:, :self.lnc],
                                     self.w[:, k, self.nhp * 128:
                                            (self.nhp + 1) * 128],
                                     xt[:, k, self.l0c:self.l0c + self.lnc],
                                     start=(k == 0), stop=(k == KC - 1))

                def evict(self):
                    if self.is_q:
                        evict_q(self.ps, self.nhp, self.l0c, self.lnc)
                    else:
                        evict_k(self.ps, self.nhp, self.l0c, self.lnc)

            tasks = []  # (deadline window, thunk)
            for nhp in range(1, HP):
                for l0c, lnc in L_CHUNKS:  # K spans full seq: due at c0
                    ch = Chain(wk, nhp, l0c, lnc, False)
                    for k in range(KC):
                        tasks.append((3 * nhp, (lambda c=ch, kk=k: c.step(kk))))
                    tasks.append((3 * nhp, (lambda c=ch: c.evict())))
                for ci in range(3):  # Q is chunk-local
                    l0c, lnc = L_CHUNKS[ci]
                    ch = Chain(wq, nhp, l0c, lnc, True)
                    for k in range(KC):
                        tasks.append((3 * nhp + ci,
                                      (lambda c=ch, kk=k: c.step(kk))))
                    tasks.append((3 * nhp + ci, (lambda c=ch: c.evict())))
            tasks.sort(key=lambda t: t[0])
            ti = [0]

            def pull(n_target):
                while ti[0] < len(tasks) and n_target > 0:
                    tasks[ti[0]][1]()
                    ti[0] += 1
                    n_target -= 1

            def due(wi):
                n = 0
                while ti[0] + n < len(tasks) and tasks[ti[0] + n][0] <= wi + 1:
                    n += 1
                return n

            for hp in range(HP):
                for ci, (l0, ln) in enumerate(L_CHUNKS):
                    wi = 3 * hp + ci
                    budget = max(due(wi),
                                 min(len(tasks) - ti[0], FILL_CAP))
                    cAB = cps.tile([128, 2, 512], F32, name="cAB", tag="cAB")
                    pending = []

                    def emit_ctx():
                        si, ss, eAB = pending.pop(0)
                        for par in range(2):
                            nc.tensor.matmul(cAB[:, par, :ln],
                                             vv[:ss, si, hp, par, :],
                                             eAB[:ss, par, :ln],
                                             start=(si == 0),
                                             stop=(si == NS - 1))

                    done = 0
                    for si, (s0, ss) in enumerate(TILES):
                        stAB = sps.tile([128, 2, 512], F32, name="stAB",
                                        tag="stAB")
                        nc.tensor.matmul(stAB[:ss, 0, :ln],
                                         kt[0:64, hp, s0:s0 + ss],
                                         qt[0:64, hp, l0:l0 + ln],
                                         start=True, stop=True,
                                         tile_position=(0, 0))
                        nc.tensor.matmul(stAB[:ss, 1, :ln],
                                         kt[64:128, hp, s0:s0 + ss],
                                         qt[64:128, hp, l0:l0 + ln],
                                         start=True, stop=True,
                                         tile_position=(64, 0))
                        eAB = wp.tile([128, 2, 512], DT, name="eAB", tag="eAB")
                        nc.scalar.activation(eAB[:ss, :, :ln],
                                             stAB[:ss, :, :ln],
                                             AF.Exp, scale=0.125)
                        pending.append((si, ss, eAB))
                        if si >= CTX_DELAY:
                            emit_ctx()
                        # front-weighted: filler budget exhausted by tile ~6
                        want = min(budget, -(-budget * (si + 1) // 6)) - done
                        if want > 0:
                            pull(want)
                            done += want
                    while pending:
                        emit_ctx()
                    if budget > done:
                        pull(budget - done)

                    rcp = wr.tile([128, 2, 512], F32, name="rcp", tag="rcp")
                    nc.vector.reciprocal(rcp[0:64, 0, :ln], cAB[64:128, 0, :ln])
                    nc.vector.reciprocal(rcp[64:128, 1, :ln], cAB[0:64, 1, :ln])
                    nc.vector.tensor_mul(ostT[0:64, hp, l0:l0 + ln],
                                         cAB[0:64, 0, :ln], rcp[0:64, 0, :ln])
                    nc.vector.tensor_mul(ostT[64:128, hp, l0:l0 + ln],
                                         cAB[64:128, 1, :ln],
                                         rcp[64:128, 1, :ln])
                    nc.sync.dma_start(out_d[hp * 128:(hp + 1) * 128,
                                            l0:l0 + ln],
                                      ostT[:, hp, l0:l0 + ln])


_NC_CACHE = {}


def _build(reps=1):
    key = ("nc", reps)
    if key in _NC_CACHE:
        return _NC_CACHE[key]
    nc = bacc.Bacc("TRN2", target_bir_lowering=False, debug=False)
    xt_d = nc.dram_tensor("xt", [HID, L], DT, kind="ExternalInput")
    wq_d = nc.dram_tensor("wqt", [HID, QD], DT, kind="ExternalInput")
    wk_d = nc.dram_tensor("wkt", [HID, QD], DT, kind="ExternalInput")
    wv_d = nc.dram_tensor("wvt", [HID, QD], DT, kind="ExternalInput")
    bq_d = nc.dram_tensor("bq", [QD, 1], F32, kind="ExternalInput")
    bv_d = nc.dram_tensor("bvb", [128, QD], F32, kind="ExternalInput")
    out_d = nc.dram_tensor("out", [QD, L], F32, kind="ExternalOutput")

    with tile.TileContext(nc) as tc:
        for _ in range(reps):
            _body(nc, tc, xt_d.ap(), wq_d.ap(), wk_d.ap(), wv_d.ap(),
                  bq_d.ap(), bv_d.ap(), out_d.ap())
    nc.compile()
    _NC_CACHE[key] = nc
    return nc


def make_in_maps(hidden_states, Wq, bq, Wk, bk, Wv, bv):
    in_maps = []
    for c in range(8):
        b, g = divmod(c, 2)
        gs = slice(g * QD, (g + 1) * QD)
        in_maps.append({
            "xt": np.ascontiguousarray(hidden_states[b].T).astype(NPDT),
            "wqt": np.ascontiguousarray(Wq[gs, :].T).astype(NPDT),
            "wkt": np.ascontiguousarray(Wk[gs, :].T).astype(NPDT),
            "wvt": np.ascontiguousarray(Wv[gs, :].T).astype(NPDT),
            "bq": bq[gs].reshape(QD, 1).astype(np.float32),
            "bvb": np.ascontiguousarray(
                np.broadcast_to(bv[gs], (128, QD))).astype(np.float32),
        })
    return in_maps


LAST_RESULTS = None


def kernel(hidden_states, Wq, bq, Wk, bk, Wv, bv):
    global LAST_RESULTS
    nc = _build()
    in_maps = make_in_maps(hidden_states, Wq, bq, Wk, bk, Wv, bv)
    try:
        res = bass_utils.run_bass_kernel_spmd(
            nc, in_maps, core_ids=list(range(8)),
            trace=bool(os.environ.get("KERNEL_TRACE")),
        )
    except (ImportError, ModuleNotFoundError):
        # The axon NTFF profiling hook is absent in some containers; retry
        # with tracing disabled rather than failing the run.
        prev = os.environ.get("BASS_NEVER_TRACE")
        os.environ["BASS_NEVER_TRACE"] = "1"
        try:
            res = bass_utils.run_bass_kernel_spmd(
                nc, in_maps, core_ids=list(range(8)))
        finally:
            if prev is None:
                os.environ.pop("BASS_NEVER_TRACE", None)
            else:
                os.environ["BASS_NEVER_TRACE"] = prev
    LAST_RESULTS = res
    out = np.empty((B, L, HID), np.float32)
    for c, om in enumerate(res.results):
        b, g = divmod(c, 2)
        out[b, :, g * QD:(g + 1) * QD] = om["out"].T
    return out


# revision 12
# speedup vs baseline: 1.5121x; 1.5121x over previous
"""Dinov2 SDPA self-attention on one TRN2 chip (8 NeuronCores).

Problem: hidden_states [4, 1370, 1024], 16 heads x 64 dim, fp32.

Sharding (hybrid data/tensor parallel): core c handles batch b = c//2 and
head-group g = c%2 (8 heads = 512 hidden columns). Each core computes its
Q/K/V projections from its batch's activations and runs attention for its
8 heads; the host transposes/concatenates the per-core [512, 1370] ctx^T
outputs. No on-chip collectives needed.

Per-core algorithm (layouts transposed so softmax reductions become matmul
contractions; ALL outputs stay in transposed [dims, L] layout -- the final
[L, 512] transpose happens on the host during unshard, so no PE transposes):
  Xt = X^T in SBUF  [1024, 1370]
  Qt = Wq_g @ Xt + bq (per-partition bias)   [512, 1370]
  Kt = Wk_g @ Xt  (K bias is softmax-invariant -> dropped exactly)
  V  = X @ Wv_g^T + bv (natural layout; softmax weights sum to 1 so
       including bv here is exact)
  per head pair (hA, hB): ST = Kt_h^T-tiles @ Qt_h = scores^T [S, L]
       (contraction d=64, PE row groups 0-63/64-127)
  P^T = exp(ST/8) (ACT, fused 1/sqrt(d) scale; scores bounded ~|4|)
  "fat-ones" context: stationary for hA is [V_hA | 1*64], for hB is
       [1*64 | V_hB], so cAB[:,0] rows 0-63 = ctx^T_A, rows 64-127 =
       rowsums (64 replicated copies); cAB[:,1] mirrored. Normalization is
       then plain per-partition DVE work in the transposed layout:
       reciprocal of the sums half + tensor_mul into ostT. No transposes.

Schedule: input DMAs arrive per hidden-chunk and the Q/K/V projection
accumulators run k-major (8 concurrent PSUM banks) so the PE tracks DMA
arrival instead of serializing behind the full transfer. The attention
loop is chunk-outer with the next head-pair's Q/K projections emitted as
filler between score/ctx matmuls (keeps PE busy during exp waits), ctx
emission delayed 2 tiles behind exp so the single cAB accumulator bank
drains (DVE normalize) without stalling the next head pair, and ctx^T
output DMAs issued per (chunk, head-pair).

Matmul operands are bf16 (fp32 PSUM accumulation); normalize stays fp32.
Validated vs fp32 reference: ~3.5e-3 of absmax.
"""

import os

import numpy as np
import ml_dtypes

import concourse.mybir as mybir
import concourse.tile as tile
from concourse import bacc
from concourse import bass_utils

F32 = mybir.dt.float32
DT = mybir.dt.bfloat16
NPDT = ml_dtypes.bfloat16
AF = mybir.ActivationFunctionType

B = 4
L = 1370
HID = 1024
NH = 8            # heads per core
D = 64
QD = NH * D       # 512 projected dims per core
HP = NH // 2      # head pairs
KC = HID // 128   # contraction chunks for projections

L_CHUNKS = [(0, 512), (512, 512), (1024, 346)]
TILES = [(i * 128, min(128, L - i * 128)) for i in range((L + 127) // 128)]
NS = len(TILES)   # 11 (last tile 90)
CTX_DELAY = 3     # tiles of lag between exp and ctx emission
FILL_CAP = 17     # max proj-task steps pulled per attention window


def _body(nc, tc, xt_d, wq_d, wk_d, wv_d, bq_d, bv_d, out_d):
    with tc.tile_pool(name="persist", bufs=1) as pp:
        xt = pp.tile([128, KC, L], DT)
        wq = pp.tile([128, KC, QD], DT)
        wk = pp.tile([128, KC, QD], DT)
        wv = pp.tile([128, KC, QD], DT)
        qt = pp.tile([128, HP, L], DT)
        kt = pp.tile([128, HP, L], DT)
        vv = pp.tile([128, NS, HP, 2, 128], DT)  # fat stationary [V|1] / [1|V]
        ostT = pp.tile([128, HP, L], F32)        # ctx^T staging
        bqc = pp.tile([128, HP], F32)
        bvb = pp.tile([128, QD], F32)

        # ones halves of the fat stationaries (rowsum replication)
        nc.gpsimd.memset(vv[:, :, :, 0, D:], 1.0)
        nc.gpsimd.memset(vv[:, :, :, 1, 0:D], 1.0)

        # Input DMAs per hidden-chunk across the three DGE queues so the
        # k-major projection waves below track arrival.
        qs = [nc.sync, nc.scalar, nc.gpsimd]
        nc.gpsimd.dma_start(bqc[:, :], bq_d.rearrange("(h p) o -> p (h o)", p=128))
        nc.gpsimd.dma_start(bvb[:, :], bv_d[:, :])
        wqv = wq_d.rearrange("(k p) n -> p k n", p=128)
        wkv = wk_d.rearrange("(k p) n -> p k n", p=128)
        LH = 685
        for k in range(KC):
            r = slice(k * 128, (k + 1) * 128)
            qs[(5 * k) % 3].dma_start(xt[:, k, 0:LH], xt_d[r, 0:LH])
            qs[(5 * k + 1) % 3].dma_start(xt[:, k, LH:], xt_d[r, LH:])
            qs[(5 * k + 2) % 3].dma_start(wq[:, k, :], wqv[:, k, :])
            qs[(5 * k + 3) % 3].dma_start(wk[:, k, :], wkv[:, k, :])
            qs[(5 * k + 4) % 3].dma_start(wv[:, k, :], wv_d[r, :])

        def evict_v(vps, si, ss):
            for par in range(2):
                nc.vector.tensor_add(
                    vv[:ss, si, :, par:par + 1, par * D:par * D + D],
                    vps[:ss, :].rearrange("p (i two d) -> p i two d", two=2, d=D)[
                        :, :, par:par + 1, :],
                    bvb[:ss, :].rearrange("p (i two d) -> p i two d", two=2, d=D)[
                        :, :, par:par + 1, :],
                )

        def evict_q(qps, hp, l0, ln):
            nc.vector.tensor_scalar_add(qt[:, hp, l0:l0 + ln], qps[:, :ln],
                                        bqc[:, hp:hp + 1])

        def evict_k(kps, hp, l0, ln):
            nc.vector.tensor_copy(kt[:, hp, l0:l0 + ln], kps[:, :ln])

        # ---- phase A: projections, per-accumulator chains rotating over the
        # 8 PSUM banks (baseline-style; chains start as their chunks land) ----
        with tc.tile_pool(name="proj", bufs=1, space="PSUM") as prp:
            rot = [0]

            def bank():
                t = prp.tile([128, QD], F32, name="pps", tag=f"pa{rot[0] % 8}")
                rot[0] += 1
                return t

            for t in range(NS):
                vps = bank()
                s0, ss = TILES[t]
                for k in range(KC):
                    nc.tensor.matmul(vps[:ss, :], xt[:, k, s0:s0 + ss],
                                     wv[:, k, :], start=(k == 0),
                                     stop=(k == KC - 1))
                evict_v(vps, t, ss)
            for l0, ln in L_CHUNKS:
                qps = bank()
                for k in range(KC):
                    nc.tensor.matmul(qps[:, :ln], wq[:, k, 0:128],
                                     xt[:, k, l0:l0 + ln], start=(k == 0),
                                     stop=(k == KC - 1))
                evict_q(qps, 0, l0, ln)
                kps = bank()
                for k in range(KC):
                    nc.tensor.matmul(kps[:, :ln], wk[:, k, 0:128],
                                     xt[:, k, l0:l0 + ln], start=(k == 0),
                                     stop=(k == KC - 1))
                evict_k(kps, 0, l0, ln)

        # ---- phase B: attention with a deadline-ordered proj task queue ----
        # Projections for head pairs 1-3 run as PE filler between attention
        # matmuls, spread evenly across windows (~FILL_CAP steps each) so no
        # window leaves the PE starved while ACT streams exp. A chain's
        # results are emitted at least one full window before the window
        # whose scores read them (the deadline).
        with (
            tc.tile_pool(name="pqp", bufs=2, space="PSUM") as pqp,
            tc.tile_pool(name="sps", bufs=2, space="PSUM") as sps,
            tc.tile_pool(name="cps", bufs=1, space="PSUM") as cps,
            tc.tile_pool(name="wp", bufs=CTX_DELAY + 2) as wp,
            tc.tile_pool(name="wr", bufs=2) as wr,
        ):
            class Chain:
                def __init__(self, w, nhp, l0c, lnc, is_q):
                    self.w, self.nhp = w, nhp
                    self.l0c, self.lnc, self.is_q = l0c, lnc, is_q
                    self.ps = None

                def step(self, k):
                    if k == 0:
                        self.ps = pqp.tile([128, 512], F32, name="pqs",
                                           tag="pq")
                    nc.tensor.matmul(self.ps[:, :self.lnc],
                                     self.w[:, k, self.nhp * 128:
                                            (self.nhp + 1) * 128],
                                     xt[:, k, self.l0c:self.l0c + self.lnc],
                                     start=(k == 0), stop=(k == KC - 1))

                def evict(self):
                    if self.is_q:
                        evict_q(self.ps, self.nhp, self.l0c, self.lnc)
                    else:
                        evict_k(self.ps, self.nhp, self.l0c, self.lnc)

            tasks = []  # (deadline window, thunk)
            for nhp in range(1, HP):
                for l0c, lnc in L_CHUNKS:  # K spans full seq: due at c0
                    ch = Chain(wk, nhp, l0c, lnc, False)
                    for k in range(KC):
                        tasks.append((3 * nhp, (lambda c=ch, kk=k: c.step(kk))))
                    tasks.append((3 * nhp, (lambda c=ch: c.evict())))
                for ci in range(3):  # Q is chunk-local
                    l0c, lnc = L_CHUNKS[ci]
                    ch = Chain(wq, nhp, l0c, lnc, True)
                    for k in range(KC):
                        tasks.append((3 * nhp + ci,
                                      (lambda c=ch, kk=k: c.step(kk))))
                    tasks.append((3 * nhp + ci, (lambda c=ch: c.evict())))
            tasks.sort(key=lambda t: t[0])
            ti = [0]

            def pull(n_target):
                while ti[0] < len(tasks) and n_target > 0:
                    tasks[ti[0]][1]()
                    ti[0] += 1
                    n_target -= 1

            def due(wi):
                n = 0
                while ti[0] + n < len(tasks) and tasks[ti[0] + n][0] <= wi + 1:
                    n += 1
                return n

            for hp in range(HP):
                for ci, (l0, ln) in enumerate(L_CHUNKS):
                    wi = 3 * hp + ci
                    budget = max(due(wi),
                                 min(len(tasks) - ti[0], FILL_CAP))
                    cAB = cps.tile([128, 2, 512], F32, name="cAB", tag="cAB")
                    pending = []

                    def emit_ctx():
                        si, ss, eAB = pending.pop(0)
                        for par in range(2):
                            nc.tensor.matmul(cAB[:, par, :ln],
                                             vv[:ss, si, hp, par, :],
                                             eAB[:ss, par, :ln],
                                             start=(si == 0),
                                             stop=(si == NS - 1))

                    done = 0
                    for si, (s0, ss) in enumerate(TILES):
                        stAB = sps.tile([128, 2, 512], F32, name="stAB",
                                        tag="stAB")
                        nc.tensor.matmul(stAB[:ss, 0, :ln],
                                         kt[0:64, hp, s0:s0 + ss],
                                         qt[0:64, hp, l0:l0 + ln],
                                         start=True, stop=True,
                                         tile_position=(0, 0))
                        nc.tensor.matmul(stAB[:ss, 1, :ln],
                                         kt[64:128, hp, s0:s0 + ss],
                                         qt[64:128, hp, l0:l0 + ln],
                                         start=True, stop=True,
                                         tile_position=(64, 0))
                        eAB = wp.tile([128, 2, 512], DT, name="eAB", tag="eAB")
                        nc.scalar.activation(eAB[:ss, :, :ln],
                                             stAB[:ss, :, :ln],
                                             AF.Exp, scale=0.125)
                        pending.append((si, ss, eAB))
                        if si >= CTX_DELAY:
                            emit_ctx()
                        # front-weighted: filler budget exhausted by tile ~6
                        want = min(budget, -(-budget * (si + 1) // 6)) - done
                        if want > 0:
                            pull(want)
                            done += want
                    while pending:
                        emit_ctx()
                    if budget > done:
                        pull(budget - done)

                    rcp = wr.tile([128, 2, 512], F32, name="rcp", tag="rcp")
                    nc.vector.reciprocal(rcp[0:64, 0, :ln], cAB[64:128, 0, :ln])
                    nc.vector.reciprocal(rcp[64:128, 1, :ln], cAB[0:64, 1, :ln])
                    nc.vector.tensor_mul(ostT[0:64, hp, l0:l0 + ln],
                                         cAB[0:64, 0, :ln], rcp[0:64, 0, :ln])
                    nc.vector.tensor_mul(ostT[64:128, hp, l0:l0 + ln],
                                         cAB[64:128, 1, :ln],
                                         rcp[64:128, 1, :ln])
                    nc.sync.dma_start(out_d[hp * 128:(hp + 1) * 128,
                                            l0:l0 + ln],
                                      ostT[:, hp, l0:l0 + ln])


_NC_CACHE = {}


def _build(reps=1):
    key = ("nc", reps)
    if key in _NC_CACHE:
        return _NC_CACHE[key]
    nc = bacc.Bacc("TRN2", target_bir_lowering=False, debug=False)
    xt_d = nc.dram_tensor("xt", [HID, L], DT, kind="ExternalInput")
    wq_d = nc.dram_tensor("wqt", [HID, QD], DT, kind="ExternalInput")
    wk_d = nc.dram_tensor("wkt", [HID, QD], DT, kind="ExternalInput")
    wv_d = nc.dram_tensor("wvt", [HID, QD], DT, kind="ExternalInput")
    bq_d = nc.dram_tensor("bq", [QD, 1], F32, kind="ExternalInput")
    bv_d = nc.dram_tensor("bvb", [128, QD], F32, kind="ExternalInput")
    out_d = nc.dram_tensor("out", [QD, L], F32, kind="ExternalOutput")

    with tile.TileContext(nc) as tc:
        for _ in range(reps):
            _body(nc, tc, xt_d.ap(), wq_d.ap(), wk_d.ap(), wv_d.ap(),
                  bq_d.ap(), bv_d.ap(), out_d.ap())
    nc.compile()
    _NC_CACHE[key] = nc
    return nc


def make_in_maps(hidden_states, Wq, bq, Wk, bk, Wv, bv):
    in_maps = []
    for c in range(8):
        b, g = divmod(c, 2)
        gs = slice(g * QD, (g + 1) * QD)
        in_maps.append({
            "xt": np.ascontiguousarray(hidden_states[b].T).astype(NPDT),
            "wqt": np.ascontiguousarray(Wq[gs, :].T).astype(NPDT),
            "wkt": np.ascontiguousarray(Wk[gs, :].T).astype(NPDT),
            "wvt": np.ascontiguousarray(Wv[gs, :].T).astype(NPDT),
            "bq": bq[gs].reshape(QD, 1).astype(np.float32),
            "bvb": np.ascontiguousarray(
                np.broadcast_to(bv[gs], (128, QD))).astype(np.float32),
        })
    return in_maps


LAST_RESULTS = None


def kernel(hidden_states, Wq, bq, Wk, bk, Wv, bv):
    global LAST_RESULTS
    nc = _build()
    in_maps = make_in_maps(hidden_states, Wq, bq, Wk, bk, Wv, bv)
    try:
        res = bass_utils.run_bass_kernel_spmd(
            nc, in_maps, core_ids=list(range(8)),
            trace=bool(os.environ.get("KERNEL_TRACE")),
        )
    except (ImportError, ModuleNotFoundError):
        # The axon NTFF profiling hook is absent in some containers; retry
        # with tracing disabled rather than failing the run.
        prev = os.environ.get("BASS_NEVER_TRACE")
        os.environ["BASS_NEVER_TRACE"] = "1"
        try:
            res = bass_utils.run_bass_kernel_spmd(
                nc, in_maps, core_ids=list(range(8)))
        finally:
            if prev is None:
                os.environ.pop("BASS_NEVER_TRACE", None)
            else:
                os.environ["BASS_NEVER_TRACE"] = prev
    LAST_RESULTS = res
    out = np.empty((B, L, HID), np.float32)
    for c, om in enumerate(res.results):
        b, g = divmod(c, 2)
        out[b, :, g * QD:(g + 1) * QD] = om["out"].T
    return out


# revision 13
# speedup vs baseline: 1.5703x; 1.0385x over previous
"""Dinov2 SDPA self-attention on one TRN2 chip (8 NeuronCores).

Problem: hidden_states [4, 1370, 1024], 16 heads x 64 dim, fp32.

Sharding (hybrid data/tensor parallel): core c handles batch b = c//2 and
head-group g = c%2 (8 heads = 512 hidden columns). Each core computes its
Q/K/V projections from its batch's activations and runs attention for its
8 heads; the host transposes/concatenates the per-core [512, 1370] ctx^T
outputs. No on-chip collectives needed.

Per-core algorithm (layouts transposed so softmax reductions become matmul
contractions; ALL outputs stay in transposed [dims, L] layout -- the final
[L, 512] transpose happens on the host during unshard, so no PE transposes):
  Xt = X^T in SBUF  [1024, 1370]
  Qt = Wq_g @ Xt + bq (per-partition bias)   [512, 1370]
  Kt = Wk_g @ Xt  (K bias is softmax-invariant -> dropped exactly)
  V  = X @ Wv_g^T + bv (natural layout; softmax weights sum to 1 so
       including bv here is exact)
  per head pair (hA, hB): ST = Kt_h^T-tiles @ Qt_h = scores^T [S, L]
       (contraction d=64, PE row groups 0-63/64-127)
  P^T = exp(ST/8) (ACT, fused 1/sqrt(d) scale; scores bounded ~|4|)
  "fat-ones" context: stationary for hA is [V_hA | 1*64], for hB is
       [1*64 | V_hB], so cAB[:,0] rows 0-63 = ctx^T_A, rows 64-127 =
       rowsums (64 replicated copies); cAB[:,1] mirrored. Normalization is
       then plain per-partition DVE work in the transposed layout:
       reciprocal of the sums half + tensor_mul into ostT. No transposes.

Schedule: input DMAs arrive per hidden-chunk and the Q/K/V projection
accumulators run k-major (8 concurrent PSUM banks) so the PE tracks DMA
arrival instead of serializing behind the full transfer. The attention
loop is chunk-outer with the next head-pair's Q/K projections emitted as
filler between score/ctx matmuls (keeps PE busy during exp waits), ctx
emission delayed 2 tiles behind exp so the single cAB accumulator bank
drains (DVE normalize) without stalling the next head pair, and ctx^T
output DMAs issued per (chunk, head-pair).

Matmul operands are bf16 (fp32 PSUM accumulation); normalize stays fp32.
Validated vs fp32 reference: ~3.5e-3 of absmax.
"""

import os

import numpy as np
import ml_dtypes

import concourse.mybir as mybir
import concourse.tile as tile
from concourse import bacc
from concourse import bass_utils

F32 = mybir.dt.float32
DT = mybir.dt.bfloat16
NPDT = ml_dtypes.bfloat16
AF = mybir.ActivationFunctionType

B = 4
L = 1370
HID = 1024
NH = 8            # heads per core
D = 64
QD = NH * D       # 512 projected dims per core
HP = NH // 2      # head pairs
KC = HID // 128   # contraction chunks for projections

L_CHUNKS = [(0, 512), (512, 512), (1024, 346)]
TILES = [(i * 128, min(128, L - i * 128)) for i in range((L + 127) // 128)]
NS = len(TILES)   # 11 (last tile 90)
CTX_DELAY = 3     # tiles of lag between exp and ctx emission
FILL_CAP = 17     # max proj-task steps pulled per attention window


def _body(nc, tc, xt_d, wq_d, wk_d, wv_d, bq_d, bv_d, out_d):
    with tc.tile_pool(name="persist", bufs=1) as pp:
        xt = pp.tile([128, KC, L], DT)
        wq = pp.tile([128, KC, QD], DT)
        wk = pp.tile([128, KC, QD], DT)
        wv = pp.tile([128, KC, QD], DT)
        qt = pp.tile([128, HP, L], DT)
        kt = pp.tile([128, HP, L], DT)
        vv = pp.tile([128, NS, HP, 2, 128], DT)  # fat stationary [V|1] / [1|V]
        ostT = pp.tile([128, HP, L], F32)        # ctx^T staging
        bqc = pp.tile([128, HP], F32)
        bvb = pp.tile([128, QD], F32)

        # ones halves of the fat stationaries (rowsum replication)
        nc.gpsimd.memset(vv[:, :, :, 0, D:], 1.0)
        nc.gpsimd.memset(vv[:, :, :, 1, 0:D], 1.0)

        # Input DMAs per hidden-chunk across the three DGE queues so the
        # k-major projection waves below track arrival.
        qs = [nc.sync, nc.scalar, nc.gpsimd]
        nc.gpsimd.dma_start(bqc[:, :], bq_d.rearrange("(h p) o -> p (h o)", p=128))
        nc.gpsimd.dma_start(bvb[:, :], bv_d[:, :])
        wqv = wq_d.rearrange("(k p) n -> p k n", p=128)
        wkv = wk_d.rearrange("(k p) n -> p k n", p=128)
        for k in range(KC):
            r = slice(k * 128, (k + 1) * 128)
            qs[(4 * k) % 3].dma_start(xt[:, k, :], xt_d[r, :])
            qs[(4 * k + 1) % 3].dma_start(wv[:, k, :], wv_d[r, :])
            qs[(4 * k + 2) % 3].dma_start(wq[:, k, :], wqv[:, k, :])
            qs[(4 * k + 3) % 3].dma_start(wk[:, k, :], wkv[:, k, :])

        def evict_v(vps, si, ss):
            for par in range(2):
                nc.vector.tensor_add(
                    vv[:ss, si, :, par:par + 1, par * D:par * D + D],
                    vps[:ss, :].rearrange("p (i two d) -> p i two d", two=2, d=D)[
                        :, :, par:par + 1, :],
                    bvb[:ss, :].rearrange("p (i two d) -> p i two d", two=2, d=D)[
                        :, :, par:par + 1, :],
                )

        def evict_q(qps, hp, l0, ln):
            nc.vector.tensor_scalar_add(qt[:, hp, l0:l0 + ln], qps[:, :ln],
                                        bqc[:, hp:hp + 1])

        def evict_k(kps, hp, l0, ln):
            nc.vector.tensor_copy(kt[:, hp, l0:l0 + ln], kps[:, :ln])

        # ---- phase A: projections, per-accumulator chains rotating over the
        # 8 PSUM banks (baseline-style; chains start as their chunks land) ----
        with tc.tile_pool(name="proj", bufs=1, space="PSUM") as prp:
            rot = [0]

            def bank():
                t = prp.tile([128, QD], F32, name="pps", tag=f"pa{rot[0] % 8}")
                rot[0] += 1
                return t

            for t in range(NS):
                vps = bank()
                s0, ss = TILES[t]
                for k in range(KC):
                    nc.tensor.matmul(vps[:ss, :], xt[:, k, s0:s0 + ss],
                                     wv[:, k, :], start=(k == 0),
                                     stop=(k == KC - 1))
                evict_v(vps, t, ss)
            for l0, ln in L_CHUNKS:
                qps = bank()
                for k in range(KC):
                    nc.tensor.matmul(qps[:, :ln], wq[:, k, 0:128],
                                     xt[:, k, l0:l0 + ln], start=(k == 0),
                                     stop=(k == KC - 1))
                evict_q(qps, 0, l0, ln)
                kps = bank()
                for k in range(KC):
                    nc.tensor.matmul(kps[:, :ln], wk[:, k, 0:128],
                                     xt[:, k, l0:l0 + ln], start=(k == 0),
                                     stop=(k == KC - 1))
                evict_k(kps, 0, l0, ln)

        # ---- phase B: attention with a deadline-ordered proj task queue ----
        # Projections for head pairs 1-3 run as PE filler between attention
        # matmuls, spread evenly across windows (~FILL_CAP steps each) so no
        # window leaves the PE starved while ACT streams exp. A chain's
        # results are emitted at least one full window before the window
        # whose scores read them (the deadline).
        with (
            tc.tile_pool(name="pqp", bufs=2, space="PSUM") as pqp,
            tc.tile_pool(name="sps", bufs=2, space="PSUM") as sps,
            tc.tile_pool(name="cps", bufs=1, space="PSUM") as cps,
            tc.tile_pool(name="wp", bufs=CTX_DELAY + 2) as wp,
            tc.tile_pool(name="wr", bufs=2) as wr,
        ):
            class Chain:
                def __init__(self, w, nhp, l0c, lnc, is_q):
                    self.w, self.nhp = w, nhp
                    self.l0c, self.lnc, self.is_q = l0c, lnc, is_q
                    self.ps = None

                def step(self, k):
                    if k == 0:
                        self.ps = pqp.tile([128, 512], F32, name="pqs",
                                           tag="pq")
                    nc.tensor.matmul(self.ps[:, :self.lnc],
                                     self.w[:, k, self.nhp * 128:
                                            (self.nhp + 1) * 128],
                                     xt[:, k, self.l0c:self.l0c + self.lnc],
                                     start=(k == 0), stop=(k == KC - 1))

                def evict(self):
                    if self.is_q:
                        evict_q(self.ps, self.nhp, self.l0c, self.lnc)
                    else:
                        evict_k(self.ps, self.nhp, self.l0c, self.lnc)

            tasks = []  # (deadline window, thunk)
            for nhp in range(1, HP):
                for l0c, lnc in L_CHUNKS:  # K spans full seq: due at c0
                    ch = Chain(wk, nhp, l0c, lnc, False)
                    for k in range(KC):
                        tasks.append((3 * nhp, (lambda c=ch, kk=k: c.step(kk))))
                    tasks.append((3 * nhp, (lambda c=ch: c.evict())))
                for ci in range(3):  # Q is chunk-local
                    l0c, lnc = L_CHUNKS[ci]
                    ch = Chain(wq, nhp, l0c, lnc, True)
                    for k in range(KC):
                        tasks.append((3 * nhp + ci,
                                      (lambda c=ch, kk=k: c.step(kk))))
                    tasks.append((3 * nhp + ci, (lambda c=ch: c.evict())))
            tasks.sort(key=lambda t: t[0])
            ti = [0]

            def pull(n_target):
                while ti[0] < len(tasks) and n_target > 0:
                    tasks[ti[0]][1]()
                    ti[0] += 1
                    n_target -= 1

            def due(wi):
                n = 0
                while ti[0] + n < len(tasks) and tasks[ti[0] + n][0] <= wi + 1:
                    n += 1
                return n

            for hp in range(HP):
                for ci, (l0, ln) in enumerate(L_CHUNKS):
                    wi = 3 * hp + ci
                    budget = max(due(wi),
                                 min(len(tasks) - ti[0], FILL_CAP))
                    cAB = cps.tile([128, 2, 512], F32, name="cAB", tag="cAB")
                    pending = []

                    def emit_ctx():
                        si, ss, eAB = pending.pop(0)
                        for par in range(2):
                            nc.tensor.matmul(cAB[:, par, :ln],
                                             vv[:ss, si, hp, par, :],
                                             eAB[:ss, par, :ln],
                                             start=(si == 0),
                                             stop=(si == NS - 1))

                    done = 0
                    for si, (s0, ss) in enumerate(TILES):
                        stAB = sps.tile([128, 2, 512], F32, name="stAB",
                                        tag="stAB")
                        nc.tensor.matmul(stAB[:ss, 0, :ln],
                                         kt[0:64, hp, s0:s0 + ss],
                                         qt[0:64, hp, l0:l0 + ln],
                                         start=True, stop=True,
                                         tile_position=(0, 0))
                        nc.tensor.matmul(stAB[:ss, 1, :ln],
                                         kt[64:128, hp, s0:s0 + ss],
                                         qt[64:128, hp, l0:l0 + ln],
                                         start=True, stop=True,
                                         tile_position=(64, 0))
                        eAB = wp.tile([128, 2, 512], DT, name="eAB", tag="eAB")
                        nc.scalar.activation(eAB[:ss, :, :ln],
                                             stAB[:ss, :, :ln],
                                             AF.Exp, scale=0.125)
                        pending.append((si, ss, eAB))
                        if si >= CTX_DELAY:
                            emit_ctx()
                        # front-weighted: filler budget exhausted by tile ~6
                        want = min(budget, -(-budget * (si + 1) // 6)) - done
                        if want > 0:
                            pull(want)
                            done += want
                    while pending:
                        emit_ctx()
                    if budget > done:
                        pull(budget - done)

                    rcp = wr.tile([128, 2, 512], F32, name="rcp", tag="rcp")
                    nc.vector.reciprocal(rcp[0:64, 0, :ln], cAB[64:128, 0, :ln])
                    nc.vector.reciprocal(rcp[64:128, 1, :ln], cAB[0:64, 1, :ln])
                    nc.vector.tensor_mul(ostT[0:64, hp, l0:l0 + ln],
                                         cAB[0:64, 0, :ln], rcp[0:64, 0, :ln])
                    nc.vector.tensor_mul(ostT[64:128, hp, l0:l0 + ln],
                                         cAB[64:128, 1, :ln],
                                         rcp[64:128, 1, :ln])
                    nc.sync.dma_start(out_d[hp * 128:(hp + 1) * 128,
                                            l0:l0 + ln],
                                      ostT[:, hp, l0:l0 + ln])


_NC_CACHE = {}


def _build(reps=1):
    key = ("nc", reps)
    if key in _NC_CACHE:
        return _NC_CACHE[key]
    nc = bacc.Bacc("TRN2", target_bir_lowering=False, debug=False)
    xt_d = nc.dram_tensor("xt", [HID, L], DT, kind="ExternalInput")
    wq_d = nc.dram_tensor("wqt", [HID, QD], DT, kind="ExternalInput")
    wk_d = nc.dram_tensor("wkt", [HID, QD], DT, kind="ExternalInput")
    wv_d = nc.dram_tensor("wvt", [HID, QD], DT, kind="ExternalInput")
    bq_d = nc.dram_tensor("bq", [QD, 1], F32, kind="ExternalInput")
    bv_d = nc.dram_tensor("bvb", [128, QD], F32, kind="ExternalInput")
    out_d = nc.dram_tensor("out", [QD, L], F32, kind="ExternalOutput")

    with tile.TileContext(nc) as tc:
        for _ in range(reps):
            _body(nc, tc, xt_d.ap(), wq_d.ap(), wk_d.ap(), wv_d.ap(),
                  bq_d.ap(), bv_d.ap(), out_d.ap())
    nc.compile()
    _NC_CACHE[key] = nc
    return nc


def make_in_maps(hidden_states, Wq, bq, Wk, bk, Wv, bv):
    in_maps = []
    for c in range(8):
        b, g = divmod(c, 2)
        gs = slice(g * QD, (g + 1) * QD)
        in_maps.append({
            "xt": np.ascontiguousarray(hidden_states[b].T).astype(NPDT),
            "wqt": np.ascontiguousarray(Wq[gs, :].T).astype(NPDT),
            "wkt": np.ascontiguousarray(Wk[gs, :].T).astype(NPDT),
            "wvt": np.ascontiguousarray(Wv[gs, :].T).astype(NPDT),
            "bq": bq[gs].reshape(QD, 1).astype(np.float32),
            "bvb": np.ascontiguousarray(
                np.broadcast_to(bv[gs], (128, QD))).astype(np.float32),
        })
    return in_maps


LAST_RESULTS = None


def kernel(hidden_states, Wq, bq, Wk, bk, Wv, bv):
    global LAST_RESULTS
    nc = _build()
    in_maps = make_in_maps(hidden_states, Wq, bq, Wk, bk, Wv, bv)
    try:
        res = bass_utils.run_bass_kernel_spmd(
            nc, in_maps, core_ids=list(range(8)),
            trace=bool(os.environ.get("KERNEL_TRACE")),
        )
    except (ImportError, ModuleNotFoundError):
        # The axon NTFF profiling hook is absent in some containers; retry
        # with tracing disabled rather than failing the run.
        prev = os.environ.get("BASS_NEVER_TRACE")
        os.environ["BASS_NEVER_TRACE"] = "1"
        try:
            res = bass_utils.run_bass_kernel_spmd(
                nc, in_maps, core_ids=list(range(8)))
        finally:
            if prev is None:
                os.environ.pop("BASS_NEVER_TRACE", None)
            else:
                os.environ["BASS_NEVER_TRACE"] = prev
    LAST_RESULTS = res
    out = np.empty((B, L, HID), np.float32)
    for c, om in enumerate(res.results):
        b, g = divmod(c, 2)
        out[b, :, g * QD:(g + 1) * QD] = om["out"].T
    return out
